# revision 1
# baseline (speedup 1.0000x reference)
"""Trainium2 Bass kernel for the attention layer:

    f = wf@x+bf; g = wg@x+bg; h = wh@x+bh            (1x1 convs, Ci=32)
    attn = softmax(f^T g, axis=-1)                   (per batch, N=4096)
    out = (wv @ (h @ attn^T) + bv) * gamma + x

Sharding: 8 cores = 4 batches x 2 query-halves (2048 queries each).
Each core receives the full (256, 4096) batch slice with its query half
permuted to the front, so the SPMD program uses fixed offsets.

Per-core dataflow (matmuls fp32r, PSUM fp32 accumulate):
  - warm-up: a dense block of dummy matmuls at t=0 so the PE HAM clock
    gate reaches 8/8 before the real work, plus a dummy exp to pull the
    ACT table load forward.
  - f/g are computed replicated onto 4 partition strips (host-replicated
    wf^T/wg^T with M=128), so the K=32 logits matmuls can be row-packed
    with tile_position: consecutive key chunks run concurrently in PE
    row bands, each writing its own PSUM bank.
  - hT (4096, 32) k-major blocks: lhsT=x k-chunk, rhs=wh^T.
  - per 512-query chunk: 32 k-chunk matmuls logitsT = g^T f (k on
    partitions, row-packed) -> ACT exp PSUM->SBUF (1024 wide) -> 32
    k-chunk accumulation rounds, each round two CONCURRENT column-tiled
    matmuls into one PSUM bank: rows 0-31 accumulate the softmax
    denominator (ones stationary), rows 32-63 accumulate x0 = h@attn^T
    (hT stationary). Reciprocal of row 0, GPSIMD partition-broadcast,
    multiply -> x0a; project with wv*gamma; bias (bv+wv@bh folded on
    host) + residual fused in one scalar_tensor_tensor; DMA out.
"""

import os
import numpy as np
import ml_dtypes

import concourse.bass as bass
import concourse.mybir as mybir
import concourse.tile as tile
from concourse import bacc
from concourse.bass import ts
from concourse.bass_utils import run_bass_kernel_spmd

F32 = mybir.dt.float32
F32R = mybir.dt.float32r
BF16 = mybir.dt.bfloat16
EXP = mybir.ActivationFunctionType.Exp
ADD = mybir.AluOpType.add

B, C, W, H = 4, 256, 64, 64
N = W * H            # 4096 keys/queries per batch
CI = 32              # inner channels
NCORES = 8
NQ = N // 2          # queries per core
QC = 512             # query chunk = one fp32 PSUM bank
NQC = NQ // QC       # 4 query chunks per core
KC = 128             # key chunk = partition dim
NKC = N // KC        # 32 key chunks
GRP = 2              # key chunks per ACT exp group (PSUM banks per tile)
NWARM = 8            # dummy fp32 matmuls to warm the PE clock gate

# Trace knob for test harnesses: set kernel.TRACE = True to profile.
TRACE = False
LAST_EXEC_NS = None

_cached_nc = None


def _mm(nc, out, lhsT, rhs, start, stop, tile_position=None):
    nc.tensor.matmul(out, lhsT=lhsT, rhs=rhs, start=start, stop=stop,
                     tile_position=tile_position)


def _build():
    nc = bacc.Bacc(
        "TRN2", target_bir_lowering=False, debug=False, num_devices=NCORES
    )
    x_d = nc.dram_tensor("x", (C, N), F32R, kind="ExternalInput").ap()
    wfT_d = nc.dram_tensor("wfT", (C, 128), F32R, kind="ExternalInput").ap()
    wgT_d = nc.dram_tensor("wgT", (C, 128), F32R, kind="ExternalInput").ap()
    whT_d = nc.dram_tensor("whT", (C, CI), BF16, kind="ExternalInput").ap()
    xbf_d = nc.dram_tensor("xbf", (C, N), BF16, kind="ExternalInput").ap()
    wvT_d = nc.dram_tensor("wvT", (CI + 1, C), F32R, kind="ExternalInput").ap()
    bf_d = nc.dram_tensor("bf", (128, 1), F32, kind="ExternalInput").ap()
    bg_d = nc.dram_tensor("bg", (128, 1), F32, kind="ExternalInput").ap()
    out_d = nc.dram_tensor("out", (C, NQ), F32, kind="ExternalOutput").ap()

    xr = x_d.rearrange("(cc p) n -> p cc n", p=128)
    outr = out_d.rearrange("(oc p) n -> p oc n", p=128)

    with tile.TileContext(nc) as tc:
        with (
            tc.tile_pool(name="consts", bufs=1) as consts,
            tc.tile_pool(name="data", bufs=1) as data,
            tc.tile_pool(name="eTp", bufs=6) as eTp,
            tc.tile_pool(name="smallp", bufs=2) as smallp,
            tc.tile_pool(name="outp", bufs=3) as outp,
            tc.tile_pool(name="pl", bufs=2, space="PSUM") as pl,
            tc.tile_pool(name="pp", bufs=2, space="PSUM") as pp,
            tc.tile_pool(name="px0", bufs=2, space="PSUM") as px0,
        ):
            # ---- PE + ACT warm-up (overlaps the input DMAs) ----
            scratch = consts.tile([128, QC], F32)
            nc.vector.memset(scratch, 0.0)
            wps = pp.tile([128, QC], F32, tag="pp")
            for i in range(NWARM):
                nc.tensor.matmul(
                    wps, lhsT=scratch[:, 0:128], rhs=scratch,
                    start=True, stop=True, skip_group_check=True,
                )
            scratch2 = consts.tile([1, 8], F32)
            nc.scalar.activation(
                out=scratch2, in_=scratch[0:1, 0:8], func=EXP
            )

            # ---- constants ----
            wfT_sb = consts.tile([128, 2, 128], F32R)
            nc.sync.dma_start(
                out=wfT_sb, in_=wfT_d.rearrange("(cc p) o -> p cc o", p=128)
            )
            wgT_sb = consts.tile([128, 2, 128], F32R)
            nc.sync.dma_start(
                out=wgT_sb, in_=wgT_d.rearrange("(cc p) o -> p cc o", p=128)
            )
            whT_sb = consts.tile([128, 2, CI], BF16)
            nc.sync.dma_start(
                out=whT_sb, in_=whT_d.rearrange("(cc p) o -> p cc o", p=128)
            )
            wvT_sb = consts.tile([CI + 1, 2, 128], F32R)
            nc.sync.dma_start(
                out=wvT_sb, in_=wvT_d.rearrange("p (oc m) -> p oc m", oc=2)
            )
            bf_sb = consts.tile([128, 1], F32)
            nc.sync.dma_start(out=bf_sb, in_=bf_d)
            bg_sb = consts.tile([128, 1], F32)
            nc.sync.dma_start(out=bg_sb, in_=bg_d)
            ones_sb = consts.tile([128, 1], F32)
            nc.vector.memset(ones_sb, 1.0)
            scratchR = consts.tile([128, QC], F32R)
            nc.vector.tensor_copy(scratchR, scratch)

            # ---- x (fp32 for f/g/residual, bf16 for the hT matmuls) ----
            x_sb = data.tile([128, 2, N], F32R)
            xbf_sb = data.tile([128, 2, N], BF16)
            xbfr = xbf_d.rearrange("(cc p) n -> p cc n", p=128)
            for s in range(4):
                nc.sync.dma_start(
                    out=x_sb[:, :, ts(s, N // 4)], in_=xr[:, :, ts(s, N // 4)]
                )
                nc.sync.dma_start(
                    out=xbf_sb[:, :, ts(s, N // 4)],
                    in_=xbfr[:, :, ts(s, N // 4)],
                )

            # ---- f, g (replicated on 4 strips), hT ----
            f_sb = data.tile([128, NQ], F32R)
            g_sb = data.tile([128, N], F32R)
            hT_sb = data.tile([128, NKC, CI + 1], F32R)
            nc.vector.tensor_copy(
                hT_sb[:, :, 0:1], ones_sb.to_broadcast([128, NKC, 1])
            )

            def emit_f(j):
                ps = pp.tile([128, QC], F32, tag="pp", name=f"psf{j}")
                for cc in range(2):
                    _mm(nc, ps, wfT_sb[:, cc, :],
                        x_sb[:, cc, ts(j, QC)], cc == 0, cc == 1)
                nc.vector.tensor_scalar_add(
                    f_sb[:, ts(j, QC)], ps, bf_sb
                )

            def emit_g(j):
                ps = pp.tile([128, QC], F32, tag="pp", name=f"psg{j}")
                for cc in range(2):
                    _mm(nc, ps, wgT_sb[:, cc, :],
                        x_sb[:, cc, ts(j, QC)], cc == 0, cc == 1)
                nc.vector.tensor_scalar_add(
                    g_sb[:, ts(j, QC)], ps, bg_sb
                )

            def emit_hT(kc):
                ps = pp.tile([128, QC], F32, tag="pp", name=f"psh{kc}")
                for cc in range(2):
                    _mm(nc, ps[:, 0:CI], xbf_sb[:, cc, ts(kc, KC)],
                        whT_sb[:, cc, :], cc == 0, cc == 1)
                nc.vector.tensor_copy(hT_sb[:, kc, 1 : CI + 1], ps[:, 0:CI])

            # f/g/hT are emitted just-in-time inside chunk 0's group
            # loop below, so the PE's in-order stream interleaves them
            # with chunk 0's logits/x0 work instead of running the whole
            # phase serially up front.
            emit_f(0)

            # ---- main loop over query chunks ----
            for qi in range(NQC):
                # row 0: softmax denominator (ones column in hT);
                # rows 1-32: x0 channels.
                x0 = px0.tile([CI + 1, QC], F32)
                x0q = []
                for g0 in range(0, NKC, GRP):
                    if qi == 0:
                        if g0 % 4 == 0:
                            emit_g(g0 // 4)
                        for kc in range(g0, g0 + GRP):
                            emit_hT(kc)
                    ps = pl.tile([128, GRP, QC], F32, tag="lg")
                    eT = eTp.tile([128, GRP, QC], F32R)
                    for j in range(GRP):
                        kc = g0 + j
                        # row-packed: strip kc%4 holds its own copy of
                        # g/f, so adjacent matmuls execute concurrently
                        # in different PE row bands.
                        s = kc % 4
                        sl = slice(32 * s, 32 * (s + 1))
                        nc.tensor.matmul(
                            ps[:, j, :],
                            lhsT=g_sb[sl, ts(kc, KC)],
                            rhs=f_sb[sl, ts(qi, QC)],
                            start=True, stop=True,
                            tile_position=(32 * s, 0),
                        )
                    nc.scalar.activation(
                        out=eT[:, :, :], in_=ps[:, :, :], func=EXP
                    )
                    # software-pipeline the x0 stage by one group: its
                    # wait on this group's exp then overlaps the NEXT
                    # group's logits in the in-order PE stream.
                    x0q.append((g0, eT))
                    if len(x0q) > 2:
                        pg0, peT = x0q.pop(0)
                        for j in range(GRP):
                            kc = pg0 + j
                            _mm(nc, x0, hT_sb[:, kc, :], peT[:, j, :],
                                kc == 0, kc == NKC - 1)
                for pg0, peT in x0q:
                    for j in range(GRP):
                        kc = pg0 + j
                        _mm(nc, x0, hT_sb[:, kc, :], peT[:, j, :],
                            kc == 0, kc == NKC - 1)
                if qi == 0:
                    for j in range(1, NQ // QC):
                        emit_f(j)
                # softmax divide: row 0 of x0 is the denominator
                rcp = smallp.tile([1, QC], F32, tag="rcp")
                nc.vector.reciprocal(rcp, x0[0:1, :])
                rcp_b = smallp.tile([CI + 1, QC], F32, tag="rcpb")
                nc.gpsimd.partition_broadcast(rcp_b, rcp)
                x0a = smallp.tile([CI + 1, QC], F32R, tag="x0a")
                nc.vector.tensor_mul(x0a, x0, rcp_b)
                # project back to C channels; bias + residual fused
                for oc in range(2):
                    vps = pp.tile([128, QC], F32, tag="pp")
                    _mm(nc, vps, wvT_sb[:, oc, :], x0a, True, True)
                    ot = outp.tile([128, QC], F32)
                    nc.vector.tensor_add(
                        ot, vps, x_sb[:, oc, ts(qi, QC)].bitcast(F32)
                    )
                    nc.sync.dma_start(out=outr[:, oc, ts(qi, QC)], in_=ot)
                if qi < NQC - 1:
                    # dense fp32r dummy matmuls: re-warm the PE clock
                    # gate in case a stall re-throttled it this chunk.
                    wb = pp.tile([128, QC], F32, tag="pp")
                    for i in range(6):
                        nc.tensor.matmul(
                            wb, lhsT=scratchR[:, 0:128], rhs=scratchR,
                            start=True, stop=True, skip_group_check=True,
                        )

    nc.compile()
    return nc


def kernel(x, wf, bf, wg, bg, wh, bh, wv, bv, gamma):
    global _cached_nc, LAST_EXEC_NS
    if _cached_nc is None:
        _cached_nc = _build()
    nc = _cached_nc

    x = np.asarray(x, dtype=np.float32)
    wf = np.asarray(wf, dtype=np.float32)
    bf = np.asarray(bf, dtype=np.float32)
    wg = np.asarray(wg, dtype=np.float32)
    bg = np.asarray(bg, dtype=np.float32)
    wh = np.asarray(wh, dtype=np.float32)
    bh = np.asarray(bh, dtype=np.float32)
    wv = np.asarray(wv, dtype=np.float32)
    bv = np.asarray(bv, dtype=np.float32)
    g0 = float(np.asarray(gamma, dtype=np.float32).reshape(-1)[0])

    xf = np.ascontiguousarray(x.reshape(B, C, N))
    # f/g weights replicated 4x along M so f/g land replicated on the
    # four 32-partition strips (enables row-packed logits matmuls).
    wfT = np.ascontiguousarray(np.tile(wf.T, (1, 4)))     # (256, 128)
    wgT = np.ascontiguousarray(np.tile(wg.T, (1, 4)))     # (256, 128)
    whT = np.ascontiguousarray(wh.T.astype(ml_dtypes.bfloat16))
    wvT = np.empty((CI + 1, C), np.float32)               # aug: bias row 0
    wvT[0, :] = g0 * (bv + wv @ bh)
    wvT[1:, :] = g0 * wv.T
    bf4 = np.ascontiguousarray(np.tile(bf, 4).reshape(128, 1))
    bg4 = np.ascontiguousarray(np.tile(bg, 4).reshape(128, 1))

    in_maps = []
    for core in range(NCORES):
        b, half = divmod(core, 2)
        xb = xf[b]
        if half:
            xb = np.ascontiguousarray(
                np.concatenate([xb[:, NQ:], xb[:, :NQ]], axis=1)
            )
        in_maps.append(
            {"x": xb, "xbf": xb.astype(ml_dtypes.bfloat16), "wfT": wfT,
             "wgT": wgT, "whT": whT, "wvT": wvT, "bf": bf4, "bg": bg4}
        )

    res = run_bass_kernel_spmd(
        nc, in_maps, list(range(NCORES)),
        trace=TRACE or bool(os.environ.get("BASS_KERNEL_TRACE")),
    )
    LAST_EXEC_NS = res.exec_time_ns

    out = np.empty((B, C, N), np.float32)
    for core in range(NCORES):
        b, half = divmod(core, 2)
        out[b][:, half * NQ : (half + 1) * NQ] = res.results[core]["out"]
    return out.reshape(B, C, W, H)



# revision 2
# speedup vs baseline: 1.1415x; 1.1415x over previous
"""Trainium2 Bass kernel for the attention layer:

    f = wf@x+bf; g = wg@x+bg; h = wh@x+bh            (1x1 convs, Ci=32)
    attn = softmax(f^T g, axis=-1)                   (per batch, N=4096)
    out = (wv @ (h @ attn^T) + bv) * gamma + x

Sharding: 8 cores = 4 batches x 2 query-halves (2048 queries each).
Each core receives the full (256, 4096) batch slice with its query half
permuted to the front, so the SPMD program uses fixed offsets.

Per-core dataflow (all-bf16 matmuls, PSUM fp32 accumulate):
  - x arrives bf16 only (2MB); the residual is added from the bf16 copy
    (tolerance is 2e-2, bf16 rounding of x costs ~4e-3).
  - f/g are computed replicated onto 4 partition strips (host-replicated
    wf^T/wg^T with M=128), so the K=32 logits matmuls can be row-packed
    with tile_position: consecutive key chunks run concurrently in PE
    row bands, each writing its own PSUM bank.
  - exp is the elementwise bottleneck (32*2048 partition-cycles/core):
    it is split between the ACT engine (true exp, PSUM->bf16) and the
    DVE (Schraudolph fast-exp: bf16 bits = int16(l*128/ln2 + 16250),
    one tensor_scalar mult+add with int16 convert, ~3.5% rel err).
  - per 512-query chunk: 32 row-packed logits matmuls -> exp groups of
    2 k-chunks -> 32 x0-accumulation matmuls (hT stationary with a ones
    column in row 0 accumulating the softmax denominator).
  - the output projection (wv*gamma stationary, bias bv+wv@bh folded
    into a row activated by the ones row of x0a) + residual + DMA for
    chunk i are DEFERRED into chunk i+1's group loop, so the PE never
    waits on the reciprocal/broadcast/mul chain.
"""

import os
import numpy as np
import ml_dtypes

import concourse.bass as bass
import concourse.mybir as mybir
import concourse.tile as tile
from concourse import bacc
from concourse.bass import ts
from concourse.bass_utils import run_bass_kernel_spmd

F32 = mybir.dt.float32
F32R = mybir.dt.float32r
BF16 = mybir.dt.bfloat16
I16 = mybir.dt.int16
EXP = mybir.ActivationFunctionType.Exp
MUL = mybir.AluOpType.mult
ADD = mybir.AluOpType.add

B, C, W, H = 4, 256, 64, 64
N = W * H            # 4096 keys/queries per batch
CI = 32              # inner channels
NCORES = 8
NQ = N // 2          # queries per core
QC = 512             # query chunk = one fp32 PSUM bank
NQC = NQ // QC       # 4 query chunks per core
KC = 128             # key chunk = partition dim
NKC = N // KC        # 32 key chunks
GRP = 2              # key chunks per exp group (PSUM banks per tile)
NGRP = NKC // GRP    # 16 groups per chunk
PIPE = 2             # software-pipeline depth (groups) for x0 stage
NWARM = 3            # dummy bf16 matmuls to warm the PE clock gate

# Schraudolph fast-exp constants (bf16 bits = int16(l*EXP_A + EXP_B))
EXP_A = 128.0 / float(np.log(2.0))
EXP_B = 16250.0
# groups handled by DVE fast-exp (rest go to ACT true exp): 5 of 16
DVE_GROUPS = frozenset(gi for gi in range(NGRP) if gi % 3 == 2)

# Trace knob for test harnesses: set kernel.TRACE = True to profile.
TRACE = False
LAST_EXEC_NS = None

_cached_nc = None


def _mm(nc, out, lhsT, rhs, start, stop, tile_position=None):
    nc.tensor.matmul(out, lhsT=lhsT, rhs=rhs, start=start, stop=stop,
                     tile_position=tile_position)


def _build():
    nc = bacc.Bacc(
        "TRN2", target_bir_lowering=False, debug=False, num_devices=NCORES
    )
    xbf_d = nc.dram_tensor("xbf", (C, N), BF16, kind="ExternalInput").ap()
    wfT_d = nc.dram_tensor("wfT", (C, 128), BF16, kind="ExternalInput").ap()
    wgT_d = nc.dram_tensor("wgT", (C, 128), BF16, kind="ExternalInput").ap()
    whT_d = nc.dram_tensor("whT", (C, CI), BF16, kind="ExternalInput").ap()
    wvT_d = nc.dram_tensor("wvT", (CI + 1, C), BF16, kind="ExternalInput").ap()
    bf_d = nc.dram_tensor("bf", (128, 1), F32, kind="ExternalInput").ap()
    bg_d = nc.dram_tensor("bg", (128, 1), F32, kind="ExternalInput").ap()
    out_d = nc.dram_tensor("out", (C, NQ), F32, kind="ExternalOutput").ap()

    outr = out_d.rearrange("(oc p) n -> p oc n", p=128)
    xbfr = xbf_d.rearrange("(cc p) n -> p cc n", p=128)

    with tile.TileContext(nc) as tc:
        with (
            tc.tile_pool(name="consts", bufs=1) as consts,
            tc.tile_pool(name="data", bufs=1) as data,
            tc.tile_pool(name="eTp", bufs=6) as eTp,
            tc.tile_pool(name="smallp", bufs=2) as smallp,
            tc.tile_pool(name="outp", bufs=3) as outp,
            tc.tile_pool(name="pl", bufs=2, space="PSUM") as pl,
            tc.tile_pool(name="pp", bufs=2, space="PSUM") as pp,
            tc.tile_pool(name="px0", bufs=2, space="PSUM") as px0,
        ):
            # ---- PE + ACT warm-up (overlaps the input DMAs) ----
            scratch = consts.tile([128, QC], BF16)
            nc.vector.memset(scratch, 0.0)
            wps = pp.tile([128, QC], F32, tag="pp")
            for i in range(NWARM):
                nc.tensor.matmul(
                    wps, lhsT=scratch[:, 0:128], rhs=scratch,
                    start=True, stop=True, skip_group_check=True,
                )
            scratch2 = consts.tile([1, 8], F32)
            nc.scalar.activation(
                out=scratch2, in_=scratch[0:1, 0:8].bitcast(I16), func=EXP
            )

            # ---- constants ----
            wfT_sb = consts.tile([128, 2, 128], BF16)
            nc.sync.dma_start(
                out=wfT_sb, in_=wfT_d.rearrange("(cc p) o -> p cc o", p=128)
            )
            wgT_sb = consts.tile([128, 2, 128], BF16)
            nc.sync.dma_start(
                out=wgT_sb, in_=wgT_d.rearrange("(cc p) o -> p cc o", p=128)
            )
            whT_sb = consts.tile([128, 2, CI], BF16)
            nc.sync.dma_start(
                out=whT_sb, in_=whT_d.rearrange("(cc p) o -> p cc o", p=128)
            )
            wvT_sb = consts.tile([CI + 1, 2, 128], BF16)
            nc.sync.dma_start(
                out=wvT_sb, in_=wvT_d.rearrange("p (oc m) -> p oc m", oc=2)
            )
            bf_sb = consts.tile([128, 1], F32)
            nc.sync.dma_start(out=bf_sb, in_=bf_d)
            bg_sb = consts.tile([128, 1], F32)
            nc.sync.dma_start(out=bg_sb, in_=bg_d)
            ones_sb = consts.tile([128, 1], BF16)
            nc.vector.memset(ones_sb, 1.0)

            # ---- x (bf16; 8 column slices so compute starts early) ----
            xbf_sb = data.tile([128, 2, N], BF16)
            for s in range(8):
                nc.sync.dma_start(
                    out=xbf_sb[:, :, ts(s, N // 8)],
                    in_=xbfr[:, :, ts(s, N // 8)],
                )

            # ---- f, g (replicated on 4 strips), hT ----
            f_sb = data.tile([128, NQ], BF16)
            g_sb = data.tile([128, N], BF16)
            hT_sb = data.tile([128, NKC, CI + 1], BF16)
            nc.vector.tensor_copy(
                hT_sb[:, :, 0:1], ones_sb.to_broadcast([128, NKC, 1])
            )

            def emit_f(j):
                ps = pp.tile([128, QC], F32, tag="pp", name=f"psf{j}")
                for cc in range(2):
                    _mm(nc, ps, wfT_sb[:, cc, :],
                        xbf_sb[:, cc, ts(j, QC)], cc == 0, cc == 1)
                nc.vector.tensor_scalar_add(
                    f_sb[:, ts(j, QC)], ps, bf_sb
                )

            def emit_g(j):
                ps = pp.tile([128, QC], F32, tag="pp", name=f"psg{j}")
                for cc in range(2):
                    _mm(nc, ps, wgT_sb[:, cc, :],
                        xbf_sb[:, cc, ts(j, QC)], cc == 0, cc == 1)
                nc.vector.tensor_scalar_add(
                    g_sb[:, ts(j, QC)], ps, bg_sb
                )

            def emit_hT(kc):
                ps = pp.tile([128, QC], F32, tag="pp", name=f"psh{kc}")
                for cc in range(2):
                    _mm(nc, ps[:, 0:CI], xbf_sb[:, cc, ts(kc, KC)],
                        whT_sb[:, cc, :], cc == 0, cc == 1)
                nc.vector.tensor_copy(hT_sb[:, kc, 1 : CI + 1], ps[:, 0:CI])

            # deferred output projection + residual for a finished chunk
            x0a_by_chunk = {}

            def emit_out(qi):
                x0a = x0a_by_chunk.pop(qi)
                for oc in range(2):
                    vps = pp.tile([128, QC], F32, tag="pp", name=f"psv{qi}{oc}")
                    _mm(nc, vps, wvT_sb[:, oc, :], x0a, True, True)
                    ot = outp.tile([128, QC], F32)
                    nc.vector.tensor_add(
                        ot, vps, xbf_sb[:, oc, ts(qi, QC)]
                    )
                    nc.sync.dma_start(out=outr[:, oc, ts(qi, QC)], in_=ot)

            # f/g/hT are emitted just-in-time inside chunk 0's group
            # loop below, so the PE's in-order stream interleaves them
            # with chunk 0's logits/x0 work instead of running the whole
            # phase serially up front.
            emit_f(0)

            # ---- main loop over query chunks ----
            for qi in range(NQC):
                # row 0: softmax denominator (ones column in hT);
                # rows 1-32: x0 channels.
                x0 = px0.tile([CI + 1, QC], F32)
                x0q = []
                for gi, g0 in enumerate(range(0, NKC, GRP)):
                    if qi == 0:
                        if g0 % 4 == 0:
                            emit_g(g0 // 4)
                        for kc in range(g0, g0 + GRP):
                            emit_hT(kc)
                    ps = pl.tile([128, GRP, QC], F32, tag="lg")
                    eT = eTp.tile([128, GRP, QC], BF16)
                    for j in range(GRP):
                        kc = g0 + j
                        # row-packed: strip kc%4 holds its own copy of
                        # g/f, so adjacent matmuls execute concurrently
                        # in different PE row bands.
                        s = kc % 4
                        sl = slice(32 * s, 32 * (s + 1))
                        nc.tensor.matmul(
                            ps[:, j, :],
                            lhsT=g_sb[sl, ts(kc, KC)],
                            rhs=f_sb[sl, ts(qi, QC)],
                            start=True, stop=True,
                            tile_position=(32 * s, 0),
                        )
                    if gi in DVE_GROUPS:
                        # Schraudolph fast-exp on DVE: bf16 bits of
                        # exp(l) ~= int16(l*EXP_A + EXP_B)
                        nc.vector.tensor_scalar(
                            out=eT.bitcast(I16), in0=ps,
                            scalar1=EXP_A, scalar2=EXP_B,
                            op0=MUL, op1=ADD,
                        )
                    else:
                        nc.scalar.activation(out=eT, in_=ps, func=EXP)
                    # software-pipeline the x0 stage: its wait on this
                    # group's exp then overlaps the NEXT group's logits
                    # in the in-order PE stream.
                    x0q.append((g0, eT))
                    if len(x0q) > PIPE:
                        pg0, peT = x0q.pop(0)
                        for j in range(GRP):
                            kc = pg0 + j
                            _mm(nc, x0, hT_sb[:, kc, :], peT[:, j, :],
                                kc == 0, kc == NKC - 1)
                    if gi == 3 and qi > 0:
                        emit_out(qi - 1)
                for pg0, peT in x0q:
                    for j in range(GRP):
                        kc = pg0 + j
                        _mm(nc, x0, hT_sb[:, kc, :], peT[:, j, :],
                            kc == 0, kc == NKC - 1)
                if qi == 0:
                    for j in range(1, NQ // QC):
                        emit_f(j)
                # softmax divide: row 0 of x0 is the denominator
                rcp = smallp.tile([1, QC], F32, tag="rcp")
                nc.vector.reciprocal(rcp, x0[0:1, :])
                rcp_b = smallp.tile([CI + 1, QC], F32, tag="rcpb")
                nc.gpsimd.partition_broadcast(rcp_b, rcp)
                x0a = smallp.tile([CI + 1, QC], BF16, tag="x0a")
                nc.vector.tensor_mul(x0a, x0, rcp_b)
                x0a_by_chunk[qi] = x0a
            emit_out(NQC - 1)

    nc.compile()
    return nc


def kernel(x, wf, bf, wg, bg, wh, bh, wv, bv, gamma):
    global _cached_nc, LAST_EXEC_NS
    if _cached_nc is None:
        _cached_nc = _build()
    nc = _cached_nc

    x = np.asarray(x, dtype=np.float32)
    wf = np.asarray(wf, dtype=np.float32)
    bf = np.asarray(bf, dtype=np.float32)
    wg = np.asarray(wg, dtype=np.float32)
    bg = np.asarray(bg, dtype=np.float32)
    wh = np.asarray(wh, dtype=np.float32)
    bh = np.asarray(bh, dtype=np.float32)
    wv = np.asarray(wv, dtype=np.float32)
    bv = np.asarray(bv, dtype=np.float32)
    g0 = float(np.asarray(gamma, dtype=np.float32).reshape(-1)[0])

    bf16 = ml_dtypes.bfloat16
    xf = np.ascontiguousarray(x.reshape(B, C, N))
    # f/g weights replicated 4x along M so f/g land replicated on the
    # four 32-partition strips (enables row-packed logits matmuls).
    wfT = np.ascontiguousarray(np.tile(wf.T, (1, 4))).astype(bf16)
    wgT = np.ascontiguousarray(np.tile(wg.T, (1, 4))).astype(bf16)
    whT = np.ascontiguousarray(wh.T).astype(bf16)
    wvT = np.empty((CI + 1, C), np.float32)              # aug: bias row 0
    wvT[0, :] = g0 * (bv + wv @ bh)
    wvT[1:, :] = g0 * wv.T
    wvT = wvT.astype(bf16)
    bf4 = np.ascontiguousarray(np.tile(bf, 4).reshape(128, 1))
    bg4 = np.ascontiguousarray(np.tile(bg, 4).reshape(128, 1))

    in_maps = []
    for core in range(NCORES):
        b, half = divmod(core, 2)
        xb = xf[b]
        if half:
            xb = np.concatenate([xb[:, NQ:], xb[:, :NQ]], axis=1)
        in_maps.append(
            {"xbf": np.ascontiguousarray(xb.astype(bf16)), "wfT": wfT,
             "wgT": wgT, "whT": whT, "wvT": wvT, "bf": bf4, "bg": bg4}
        )

    res = run_bass_kernel_spmd(
        nc, in_maps, list(range(NCORES)),
        trace=TRACE or bool(os.environ.get("BASS_KERNEL_TRACE")),
    )
    LAST_EXEC_NS = res.exec_time_ns

    out = np.empty((B, C, N), np.float32)
    for core in range(NCORES):
        b, half = divmod(core, 2)
        out[b][:, half * NQ : (half + 1) * NQ] = res.results[core]["out"]
    return out.reshape(B, C, W, H)


# revision 3
# speedup vs baseline: 1.2527x; 1.0975x over previous
"""Trainium2 Bass kernel for the attention layer:

    f = wf@x+bf; g = wg@x+bg; h = wh@x+bh            (1x1 convs, Ci=32)
    attn = softmax(f^T g, axis=-1)                   (per batch, N=4096)
    out = (wv @ (h @ attn^T) + bv) * gamma + x

Sharding: 8 cores = 4 batches x 2 query-halves (2048 queries each).
Each core receives the full (256, 4096) batch slice with its query half
permuted to the front, so the SPMD program uses fixed offsets.

Per-core dataflow (all-bf16 matmuls, PSUM fp32 accumulate):
  - x arrives bf16 only (2MB); the residual is added from the bf16 copy
    via an IDENTITY MATMUL accumulated into the projection PSUM bank,
    so the output copy is a pure PSUM->SBUF copy (balanceable between
    ACT and DVE) instead of a DVE-only tensor_tensor add.
  - f/g are computed replicated onto 4 partition strips (host-replicated
    wf^T/wg^T with M=128), so the K=32 logits matmuls can be row-packed
    with tile_position (concurrent PE row bands).
  - exp is the elementwise bottleneck (32*2048 partition-cycles/core)
    and GPSIMD/DMA cannot touch PSUM, so every PSUM consumer is either
    ACT or DVE and must be balanced: exp groups are split ACT (true
    exp, PSUM->bf16) vs DVE (Schraudolph fast-exp: bf16 bits =
    int16(l*128/ln2 + 16250) in one tensor_scalar, ~3.5% rel err);
    PSUM->SBUF copies alternate engines; softmax divide uses
    reciprocal_approx_fast (5x the plain InstReciprocal).
  - hT (keys-major) is produced into one PSUM bank per 16 key-chunks
    ([128,16,32] fp32 = 2KB) so one big copy replaces 16 small ones.
  - per 512-query chunk: 32 row-packed logits matmuls -> exp groups of
    2 k-chunks -> 32 x0-accumulation matmuls (hT stationary with a ones
    column in row 0 accumulating the softmax denominator).
  - projection+residual+DMA for chunk i are DEFERRED into chunk i+1's
    group loop; dummy 512-row matmuls pad PE idle slots to keep the HAM
    clock gate from downshifting the PE frequency.
"""

import os
import numpy as np
import ml_dtypes

import concourse.bass as bass
import concourse.mybir as mybir
import concourse.tile as tile
from concourse import bacc
from concourse.bass import ts
from concourse.bass_utils import run_bass_kernel_spmd

F32 = mybir.dt.float32
F32R = mybir.dt.float32r
BF16 = mybir.dt.bfloat16
I16 = mybir.dt.int16
EXP = mybir.ActivationFunctionType.Exp
IDENT = mybir.ActivationFunctionType.Identity
MUL = mybir.AluOpType.mult
ADD = mybir.AluOpType.add

B, C, W, H = 4, 256, 64, 64
N = W * H            # 4096 keys/queries per batch
CI = 32              # inner channels
NCORES = 8
NQ = N // 2          # queries per core
QC = 512             # query chunk = one fp32 PSUM bank
NQC = NQ // QC       # 4 query chunks per core
KC = 128             # key chunk = partition dim
NKC = N // KC        # 32 key chunks
GRP = 2              # key chunks per exp group (PSUM banks per tile)
NGRP = NKC // GRP    # 16 groups per chunk
PIPE = 2             # software-pipeline depth (groups) for x0 stage
NWARM = 8            # dummy bf16 matmuls to warm the PE clock gate
FILL_EVERY = 2       # insert a PE filler matmul every this many groups

# Schraudolph fast-exp constants (bf16 bits = int16(l*EXP_A + EXP_B))
EXP_A = 128.0 / float(np.log(2.0))
EXP_B = 16250.0
# groups handled by DVE fast-exp (rest go to ACT true exp): 7 of 16
DVE_GROUPS = frozenset({1, 3, 5, 8, 10, 12, 14})

# Trace knob for test harnesses: set kernel.TRACE = True to profile.
TRACE = False
LAST_EXEC_NS = None

_cached_nc = None


def _mm(nc, out, lhsT, rhs, start, stop, tile_position=None):
    nc.tensor.matmul(out, lhsT=lhsT, rhs=rhs, start=start, stop=stop,
                     tile_position=tile_position)


def _build():
    nc = bacc.Bacc(
        "TRN2", target_bir_lowering=False, debug=False, num_devices=NCORES
    )
    xbf_d = nc.dram_tensor("xbf", (C, N), BF16, kind="ExternalInput").ap()
    wfT_d = nc.dram_tensor("wfT", (C, 128), BF16, kind="ExternalInput").ap()
    wgT_d = nc.dram_tensor("wgT", (C, 128), BF16, kind="ExternalInput").ap()
    whT_d = nc.dram_tensor("whT", (C, CI), BF16, kind="ExternalInput").ap()
    wvT_d = nc.dram_tensor("wvT", (CI + 1, C), BF16, kind="ExternalInput").ap()
    wid_d = nc.dram_tensor("wid", (128, 128), BF16, kind="ExternalInput").ap()
    bf_d = nc.dram_tensor("bf", (128, 1), F32, kind="ExternalInput").ap()
    bg_d = nc.dram_tensor("bg", (128, 1), F32, kind="ExternalInput").ap()
    out_d = nc.dram_tensor("out", (C, NQ), F32, kind="ExternalOutput").ap()

    outr = out_d.rearrange("(oc p) n -> p oc n", p=128)
    xbfr = xbf_d.rearrange("(cc p) n -> p cc n", p=128)

    with tile.TileContext(nc) as tc:
        with (
            tc.tile_pool(name="consts", bufs=1) as consts,
            tc.tile_pool(name="data", bufs=1) as data,
            tc.tile_pool(name="eTp", bufs=6) as eTp,
            tc.tile_pool(name="smallp", bufs=2) as smallp,
            tc.tile_pool(name="outp", bufs=3) as outp,
            tc.tile_pool(name="pl", bufs=2, space="PSUM") as pl,
            tc.tile_pool(name="pp", bufs=2, space="PSUM") as pp,
            tc.tile_pool(name="px0", bufs=1, space="PSUM") as px0,
            tc.tile_pool(name="ph", bufs=1, space="PSUM") as ph,
        ):
            # ---- PE + ACT warm-up (overlaps the input DMAs) ----
            scratch = consts.tile([128, QC], BF16)
            nc.vector.memset(scratch, 0.0)
            wps = pp.tile([128, QC], F32, tag="pp")
            for i in range(NWARM):
                nc.tensor.matmul(
                    wps, lhsT=scratch[:, 0:128], rhs=scratch,
                    start=True, stop=True, skip_group_check=True,
                )
            scratch2 = consts.tile([1, 8], F32)
            nc.scalar.activation(out=scratch2, in_=scratch[0:1, 0:8], func=EXP)

            # ---- constants ----
            wfT_sb = consts.tile([128, 2, 128], BF16)
            nc.sync.dma_start(
                out=wfT_sb, in_=wfT_d.rearrange("(cc p) o -> p cc o", p=128)
            )
            wgT_sb = consts.tile([128, 2, 128], BF16)
            nc.sync.dma_start(
                out=wgT_sb, in_=wgT_d.rearrange("(cc p) o -> p cc o", p=128)
            )
            whT_sb = consts.tile([128, 2, CI], BF16)
            nc.sync.dma_start(
                out=whT_sb, in_=whT_d.rearrange("(cc p) o -> p cc o", p=128)
            )
            wvT_sb = consts.tile([CI + 1, 2, 128], BF16)
            nc.sync.dma_start(
                out=wvT_sb, in_=wvT_d.rearrange("p (oc m) -> p oc m", oc=2)
            )
            wid_sb = consts.tile([128, 128], BF16)
            nc.sync.dma_start(out=wid_sb, in_=wid_d)
            bf_sb = consts.tile([128, 1], F32)
            nc.sync.dma_start(out=bf_sb, in_=bf_d)
            bg_sb = consts.tile([128, 1], F32)
            nc.sync.dma_start(out=bg_sb, in_=bg_d)
            ones_sb = consts.tile([128, 1], BF16)
            nc.vector.memset(ones_sb, 1.0)

            # ---- x (bf16; 8 column slices so compute starts early) ----
            xbf_sb = data.tile([128, 2, N], BF16)
            for s in range(8):
                nc.sync.dma_start(
                    out=xbf_sb[:, :, ts(s, N // 8)],
                    in_=xbfr[:, :, ts(s, N // 8)],
                )

            # ---- f, g (replicated on 4 strips), hT ----
            f_sb = data.tile([128, NQ], BF16)
            g_sb = data.tile([128, N], BF16)
            hT_sb = data.tile([128, NKC, CI + 1], BF16)
            nc.vector.tensor_copy(
                hT_sb[:, :, 0:1], ones_sb.to_broadcast([128, NKC, 1])
            )

            # alternate PSUM->SBUF copy work between ACT and DVE
            def emit_f(j, on_act):
                ps = pp.tile([128, QC], F32, tag="pp", name=f"psf{j}")
                for cc in range(2):
                    _mm(nc, ps, wfT_sb[:, cc, :],
                        xbf_sb[:, cc, ts(j, QC)], cc == 0, cc == 1)
                if on_act:
                    nc.scalar.activation(
                        out=f_sb[:, ts(j, QC)], in_=ps, func=IDENT, bias=bf_sb
                    )
                else:
                    nc.vector.tensor_scalar_add(f_sb[:, ts(j, QC)], ps, bf_sb)

            def emit_g(j, on_act):
                ps = pp.tile([128, QC], F32, tag="pp", name=f"psg{j}")
                for cc in range(2):
                    _mm(nc, ps, wgT_sb[:, cc, :],
                        xbf_sb[:, cc, ts(j, QC)], cc == 0, cc == 1)
                if on_act:
                    nc.scalar.activation(
                        out=g_sb[:, ts(j, QC)], in_=ps, func=IDENT, bias=bg_sb
                    )
                else:
                    nc.vector.tensor_scalar_add(g_sb[:, ts(j, QC)], ps, bg_sb)

            # hT production: one PSUM bank holds 16 key-chunks
            # ([128,16,32] fp32 = 2KB/partition), one big copy each.
            hT_ps = [None, None]

            def emit_hT_mm(kc):
                half, sl = divmod(kc, 16)
                if hT_ps[half] is None:
                    hT_ps[half] = ph.tile(
                        [128, 16, CI], F32, tag="ph", name=f"ph{half}"
                    )
                for cc in range(2):
                    _mm(nc, hT_ps[half][:, sl, :], xbf_sb[:, cc, ts(kc, KC)],
                        whT_sb[:, cc, :], cc == 0, cc == 1)

            def emit_hT_copy(half, on_act):
                src = hT_ps[half]
                dst = hT_sb[:, 16 * half : 16 * (half + 1), 1 : CI + 1]
                if on_act:
                    nc.scalar.copy(dst, src)
                else:
                    nc.vector.tensor_copy(dst, src)

            # deferred projection + residual + output for a finished chunk
            x0a_by_chunk = {}

            def emit_out(qi):
                x0a = x0a_by_chunk.pop(qi)
                for oc in range(2):
                    vps = pp.tile([128, QC], F32, tag="pp", name=f"psv{qi}{oc}")
                    _mm(nc, vps, wvT_sb[:, oc, :], x0a, True, False)
                    # residual: + I @ x  (identity matmul accumulate)
                    _mm(nc, vps, wid_sb, xbf_sb[:, oc, ts(qi, QC)],
                        False, True)
                    ot = outp.tile([128, QC], F32)
                    if oc == 0:
                        nc.scalar.copy(ot, vps)
                    else:
                        nc.vector.tensor_copy(ot, vps)
                    nc.sync.dma_start(out=outr[:, oc, ts(qi, QC)], in_=ot)

            # ---- chunk-0 prologue: f(0), g blocks 0-3, hT kc 0-15 ----
            emit_f(0, on_act=False)
            for s in range(4):
                emit_g(s, on_act=(s % 2 == 0))
                for kc in range(4 * s, 4 * s + 4):
                    emit_hT_mm(kc)
            emit_hT_copy(0, on_act=True)

            # B-batch hT (kc 16-31) emission schedule inside chunk 0
            b_sched = {0: [16, 17, 18], 1: [19, 20, 21], 2: [22, 23, 24],
                       3: [25, 26, 27], 4: [28, 29], 5: [30, 31]}

            # ---- main loop over query chunks ----
            for qi in range(NQC):
                # row 0: softmax denominator (ones column in hT);
                # rows 1-32: x0 channels.
                x0 = px0.tile([CI + 1, QC], F32)
                x0q = []
                for gi, g0 in enumerate(range(0, NKC, GRP)):
                    if qi == 0:
                        for kc in b_sched.get(gi, []):
                            emit_hT_mm(kc)
                        if gi == 6:
                            emit_hT_copy(1, on_act=False)
                        if gi in (7, 9, 11, 13):
                            emit_g(4 + (gi - 7) // 2, on_act=(gi % 4 == 3))
                    ps = pl.tile([128, GRP, QC], F32, tag="lg")
                    eT = eTp.tile([128, GRP, QC], BF16)
                    if gi % FILL_EVERY == 1:
                        # HAM filler: keep the PE streaming through slots
                        # where it would otherwise idle waiting on exp.
                        nc.tensor.matmul(
                            ps[:, 0, :], lhsT=scratch[:, 0:128], rhs=scratch,
                            start=True, stop=True, skip_group_check=True,
                        )
                    for j in range(GRP):
                        kc = g0 + j
                        # row-packed: strip kc%4 holds its own copy of
                        # g/f, so adjacent matmuls execute concurrently
                        # in different PE row bands.
                        s = kc % 4
                        sl = slice(32 * s, 32 * (s + 1))
                        nc.tensor.matmul(
                            ps[:, j, :],
                            lhsT=g_sb[sl, ts(kc, KC)],
                            rhs=f_sb[sl, ts(qi, QC)],
                            start=True, stop=True,
                            tile_position=(32 * s, 0),
                        )
                    if gi in DVE_GROUPS:
                        # Schraudolph fast-exp on DVE: bf16 bits of
                        # exp(l) ~= int16(l*EXP_A + EXP_B)
                        nc.vector.tensor_scalar(
                            out=eT.bitcast(I16), in0=ps,
                            scalar1=EXP_A, scalar2=EXP_B,
                            op0=MUL, op1=ADD,
                        )
                    else:
                        nc.scalar.activation(out=eT, in_=ps, func=EXP)
                    # software-pipeline the x0 stage: its wait on this
                    # group's exp then overlaps the NEXT group's logits
                    # in the in-order PE stream.
                    x0q.append((g0, eT))
                    if len(x0q) > PIPE:
                        pg0, peT = x0q.pop(0)
                        for j in range(GRP):
                            kc = pg0 + j
                            _mm(nc, x0, hT_sb[:, kc, :], peT[:, j, :],
                                kc == 0, kc == NKC - 1)
                    if gi == 3 and qi > 0:
                        emit_out(qi - 1)
                for pg0, peT in x0q:
                    for j in range(GRP):
                        kc = pg0 + j
                        _mm(nc, x0, hT_sb[:, kc, :], peT[:, j, :],
                            kc == 0, kc == NKC - 1)
                if qi == 0:
                    for j in range(1, NQ // QC):
                        emit_f(j, on_act=(j % 2 == 0))
                # softmax divide: row 0 of x0 is the denominator
                rcp = smallp.tile([1, QC], F32, tag="rcp")
                nc.vector.reciprocal_approx_fast(out=rcp, in_=x0[0:1, :])
                rcp_b = smallp.tile([CI + 1, QC], F32, tag="rcpb")
                nc.gpsimd.partition_broadcast(rcp_b, rcp)
                x0a = smallp.tile([CI + 1, QC], BF16, tag="x0a")
                nc.vector.tensor_mul(x0a, x0, rcp_b)
                x0a_by_chunk[qi] = x0a
            emit_out(NQC - 1)

    nc.compile()
    return nc


def kernel(x, wf, bf, wg, bg, wh, bh, wv, bv, gamma):
    global _cached_nc, LAST_EXEC_NS
    if _cached_nc is None:
        _cached_nc = _build()
    nc = _cached_nc

    x = np.asarray(x, dtype=np.float32)
    wf = np.asarray(wf, dtype=np.float32)
    bf = np.asarray(bf, dtype=np.float32)
    wg = np.asarray(wg, dtype=np.float32)
    bg = np.asarray(bg, dtype=np.float32)
    wh = np.asarray(wh, dtype=np.float32)
    bh = np.asarray(bh, dtype=np.float32)
    wv = np.asarray(wv, dtype=np.float32)
    bv = np.asarray(bv, dtype=np.float32)
    g0 = float(np.asarray(gamma, dtype=np.float32).reshape(-1)[0])

    bf16 = ml_dtypes.bfloat16
    xf = np.ascontiguousarray(x.reshape(B, C, N))
    # f/g weights replicated 4x along M so f/g land replicated on the
    # four 32-partition strips (enables row-packed logits matmuls).
    wfT = np.ascontiguousarray(np.tile(wf.T, (1, 4))).astype(bf16)
    wgT = np.ascontiguousarray(np.tile(wg.T, (1, 4))).astype(bf16)
    whT = np.ascontiguousarray(wh.T).astype(bf16)
    wvT = np.empty((CI + 1, C), np.float32)              # aug: bias row 0
    wvT[0, :] = g0 * (bv + wv @ bh)
    wvT[1:, :] = g0 * wv.T
    wvT = wvT.astype(bf16)
    wid = np.eye(128, dtype=np.float32).astype(bf16)
    bf4 = np.ascontiguousarray(np.tile(bf, 4).reshape(128, 1))
    bg4 = np.ascontiguousarray(np.tile(bg, 4).reshape(128, 1))

    in_maps = []
    for core in range(NCORES):
        b, half = divmod(core, 2)
        xb = xf[b]
        if half:
            xb = np.concatenate([xb[:, NQ:], xb[:, :NQ]], axis=1)
        in_maps.append(
            {"xbf": np.ascontiguousarray(xb.astype(bf16)), "wfT": wfT,
             "wgT": wgT, "whT": whT, "wvT": wvT, "wid": wid,
             "bf": bf4, "bg": bg4}
        )

    res = run_bass_kernel_spmd(
        nc, in_maps, list(range(NCORES)),
        trace=TRACE or bool(os.environ.get("BASS_KERNEL_TRACE")),
    )
    LAST_EXEC_NS = res.exec_time_ns

    out = np.empty((B, C, N), np.float32)
    for core in range(NCORES):
        b, half = divmod(core, 2)
        out[b][:, half * NQ : (half + 1) * NQ] = res.results[core]["out"]
    return out.reshape(B, C, W, H)


# revision 5
# speedup vs baseline: 1.3506x; 1.0781x over previous
"""Trainium2 Bass kernel for the attention layer:

    f = wf@x+bf; g = wg@x+bg; h = wh@x+bh            (1x1 convs, Ci=32)
    attn = softmax(f^T g, axis=-1)                   (per batch, N=4096)
    out = (wv @ (h @ attn^T) + bv) * gamma + x

Sharding: 8 cores = 4 batches x 2 query-halves (2048 queries each).
Each core receives the full (256, 4096) batch slice with its query half
permuted to the front, so the SPMD program uses fixed offsets.

Per-core dataflow (all-bf16 matmuls, PSUM fp32 accumulate):
  - x arrives bf16 only (2MB); the residual is added from the bf16 copy
    via an IDENTITY MATMUL accumulated into the projection PSUM bank,
    so the output copy is a pure PSUM->SBUF copy (balanceable between
    ACT and DVE) instead of a DVE-only tensor_tensor add.
  - exp is the elementwise bottleneck (32*2048 partition-cycles/core)
    and GPSIMD/DMA cannot touch PSUM, so every PSUM consumer is either
    ACT or DVE: exp groups alternate ACT (true exp, PSUM->bf16) and DVE
    (Schraudolph fast-exp: bf16 bits = int16(l*128/ln2 + 16250) in one
    tensor_scalar, ~3.5% rel err); PSUM->SBUF copies alternate engines;
    softmax divide uses reciprocal_approx_fast.
  - PSUM budget (8 banks): logits pool 3 bufs x 2 banks so the logits
    matmul for group g WAR-waits exp(g-3), letting both exp engines run
    fully parallel; 1 bank for projections/warmup/fillers; 1 bank for
    the x0 accumulator.
  - the whole f/g/hT prologue runs before the chunk loop, cycling
    through the logits pool buffers; hT lands 16 key-chunks per PSUM
    bank ([128,16,32] fp32 = 2KB) so one big copy replaces 16 small.
  - per 512-query chunk: 32 row-packed logits matmuls (strip kc%4,
    concurrent PE row bands) -> exp groups of 2 k-chunks -> 32
    x0-accumulation matmuls (hT stationary, ones column in row 0
    accumulating the softmax denominator).
  - projection+residual+DMA for chunk i are DEFERRED into chunk i+1's
    group loop (two halves at different groups so PSUM WARs stay off
    the PE's critical path); dummy 512-row matmuls pad PE idle slots so
    the HAM clock gate (K/N pulse gating, 1.2 vs 2.4 GHz) stays open.
"""

import os
import numpy as np
import ml_dtypes

import concourse.bass as bass
import concourse.mybir as mybir
import concourse.tile as tile
from concourse import bacc
from concourse.bass import ts
from concourse.bass_utils import run_bass_kernel_spmd

F32 = mybir.dt.float32
F32R = mybir.dt.float32r
BF16 = mybir.dt.bfloat16
I16 = mybir.dt.int16
EXP = mybir.ActivationFunctionType.Exp
IDENT = mybir.ActivationFunctionType.Identity
MUL = mybir.AluOpType.mult
ADD = mybir.AluOpType.add

B, C, W, H = 4, 256, 64, 64
N = W * H            # 4096 keys/queries per batch
CI = 32              # inner channels
NCORES = 8
NQ = N // 2          # queries per core
QC = 512             # query chunk = one fp32 PSUM bank
NQC = NQ // QC       # 4 query chunks per core
KC = 128             # key chunk = partition dim
NKC = N // KC        # 32 key chunks
GRP = 2              # key chunks per exp group (PSUM banks per tile)
NGRP = NKC // GRP    # 16 groups per chunk
PIPE = 2             # software-pipeline depth (groups) for x0 stage
NWARM = 8            # dummy bf16 matmuls to warm the PE clock gate
FILL_EVERY = 1       # PE filler matmul every this many groups
NSLICE = 4           # x DMA slices

# Schraudolph fast-exp constants (bf16 bits = int16(l*EXP_A + EXP_B))
EXP_A = 128.0 / float(np.log(2.0))
EXP_B = 16250.0
# groups handled by DVE fast-exp (rest go to ACT true exp): 7 of 16
DVE_GROUPS = frozenset({1, 3, 5, 7, 9, 11, 13})

# Trace knob for test harnesses: set kernel.TRACE = True to profile.
TRACE = False
LAST_EXEC_NS = None

_cached_nc = None


def _mm(nc, out, lhsT, rhs, start, stop, tile_position=None):
    nc.tensor.matmul(out, lhsT=lhsT, rhs=rhs, start=start, stop=stop,
                     tile_position=tile_position)


def _build():
    nc = bacc.Bacc(
        "TRN2", target_bir_lowering=False, debug=False, num_devices=NCORES
    )
    xbf_d = nc.dram_tensor("xbf", (C, N), BF16, kind="ExternalInput").ap()
    wfT_d = nc.dram_tensor("wfT", (C, 128), BF16, kind="ExternalInput").ap()
    wgT_d = nc.dram_tensor("wgT", (C, 128), BF16, kind="ExternalInput").ap()
    whT_d = nc.dram_tensor("whT", (C, CI), BF16, kind="ExternalInput").ap()
    wvT_d = nc.dram_tensor("wvT", (CI + 1, C), BF16, kind="ExternalInput").ap()
    wid_d = nc.dram_tensor("wid", (128, 128), BF16, kind="ExternalInput").ap()
    bf_d = nc.dram_tensor("bf", (128, 1), F32, kind="ExternalInput").ap()
    bg_d = nc.dram_tensor("bg", (128, 1), F32, kind="ExternalInput").ap()
    out_d = nc.dram_tensor("out", (C, NQ), F32, kind="ExternalOutput").ap()

    outr = out_d.rearrange("(oc p) n -> p oc n", p=128)
    xbfr = xbf_d.rearrange("(cc p) n -> p cc n", p=128)

    with tile.TileContext(nc) as tc:
        with (
            tc.tile_pool(name="consts", bufs=1) as consts,
            tc.tile_pool(name="data", bufs=1) as data,
            tc.tile_pool(name="eTp", bufs=6) as eTp,
            tc.tile_pool(name="smallp", bufs=2) as smallp,
            tc.tile_pool(name="outp", bufs=3) as outp,
            tc.tile_pool(name="pl", bufs=3, space="PSUM") as pl,
            tc.tile_pool(name="pp", bufs=1, space="PSUM") as pp,
            tc.tile_pool(name="px0", bufs=1, space="PSUM") as px0,
        ):
            # ---- PE + ACT warm-up (overlaps the input DMAs) ----
            scratch = consts.tile([128, QC], BF16)
            nc.vector.memset(scratch, 0.0)

            def fill(n=1):
                # HAM filler: keeps the PE streaming through slots where
                # it would otherwise idle (and re-throttle to 1.2 GHz).
                wps = pp.tile([128, QC], F32, tag="pp", name="wps")
                for _ in range(n):
                    nc.tensor.matmul(
                        wps, lhsT=scratch[:, 0:128], rhs=scratch,
                        start=True, stop=True, skip_group_check=True,
                    )

            fill(NWARM)
            scratch2 = consts.tile([1, 8], F32)
            nc.scalar.activation(out=scratch2, in_=scratch[0:1, 0:8], func=EXP)

            # ---- constants ----
            wfT_sb = consts.tile([128, 2, 128], BF16)
            nc.sync.dma_start(
                out=wfT_sb, in_=wfT_d.rearrange("(cc p) o -> p cc o", p=128)
            )
            wgT_sb = consts.tile([128, 2, 128], BF16)
            nc.sync.dma_start(
                out=wgT_sb, in_=wgT_d.rearrange("(cc p) o -> p cc o", p=128)
            )
            whT_sb = consts.tile([128, 2, CI], BF16)
            nc.sync.dma_start(
                out=whT_sb, in_=whT_d.rearrange("(cc p) o -> p cc o", p=128)
            )
            wvT_sb = consts.tile([CI + 1, 2, 128], BF16)
            nc.sync.dma_start(
                out=wvT_sb, in_=wvT_d.rearrange("p (oc m) -> p oc m", oc=2)
            )
            wid_sb = consts.tile([128, 128], BF16)
            nc.sync.dma_start(out=wid_sb, in_=wid_d)
            bf_sb = consts.tile([128, 1], F32)
            nc.sync.dma_start(out=bf_sb, in_=bf_d)
            bg_sb = consts.tile([128, 1], F32)
            nc.sync.dma_start(out=bg_sb, in_=bg_d)
            ones_sb = consts.tile([128, 1], BF16)
            nc.vector.memset(ones_sb, 1.0)

            # ---- x (bf16, 4 column slices so compute starts early) ----
            xbf_sb = data.tile([128, 2, N], BF16)
            for s in range(NSLICE):
                nc.sync.dma_start(
                    out=xbf_sb[:, :, ts(s, N // NSLICE)],
                    in_=xbfr[:, :, ts(s, N // NSLICE)],
                )

            # ---- f, g (replicated on 4 strips), hT ----
            f_sb = data.tile([128, NQ], BF16)
            g_sb = data.tile([128, N], BF16)
            hT_sb = data.tile([128, NKC, CI + 1], BF16)
            nc.vector.tensor_copy(
                hT_sb[:, :, 0:1], ones_sb.to_broadcast([128, NKC, 1])
            )

            # prologue blocks cycle through the logits pool's 3 buffers;
            # PSUM->SBUF copy work alternates between ACT and DVE.
            def emit_fg(dst, w_sb, b_sb, j, on_act):
                ps = pl.tile([128, GRP, QC], F32, tag="lg", name=f"fg{j}")
                for cc in range(2):
                    _mm(nc, ps[:, 0, :], w_sb[:, cc, :],
                        xbf_sb[:, cc, ts(j, QC)], cc == 0, cc == 1)
                if on_act:
                    nc.scalar.activation(
                        out=dst[:, ts(j, QC)], in_=ps[:, 0, :], func=IDENT,
                        bias=b_sb,
                    )
                else:
                    nc.vector.tensor_scalar_add(
                        dst[:, ts(j, QC)], ps[:, 0, :], b_sb
                    )

            # hT production: one PSUM bank holds 16 key-chunks
            # ([128,16,32] fp32 = 2KB/partition), one big copy each.
            def emit_hT(half, on_act):
                hps = pl.tile([128, 16, CI], F32, tag="lg", name=f"ph{half}")
                for sl in range(16):
                    kc = 16 * half + sl
                    for cc in range(2):
                        _mm(nc, hps[:, sl, :], xbf_sb[:, cc, ts(kc, KC)],
                            whT_sb[:, cc, :], cc == 0, cc == 1)
                dst = hT_sb[:, 16 * half : 16 * (half + 1), 1 : CI + 1]
                if on_act:
                    nc.scalar.copy(dst, hps)
                else:
                    nc.vector.tensor_copy(dst, hps)

            # ---- prologue: f (own queries), g + hT (all keys) ----
            for j in range(4):
                emit_fg(f_sb, wfT_sb, bf_sb, j, on_act=(j % 2 == 0))
                emit_fg(g_sb, wgT_sb, bg_sb, j, on_act=(j % 2 == 1))
                fill(1)
            emit_hT(0, on_act=True)
            for j in range(4, 8):
                emit_fg(g_sb, wgT_sb, bg_sb, j, on_act=(j % 2 == 1))
                fill(1)
            emit_hT(1, on_act=False)

            # deferred projection + residual + output for chunk qi
            x0a_by_chunk = {}

            def emit_out(qi, oc):
                x0a = x0a_by_chunk[qi]
                vps = pp.tile([128, QC], F32, tag="pp", name=f"psv{qi}{oc}")
                _mm(nc, vps, wvT_sb[:, oc, :], x0a, True, False)
                # residual: + I @ x  (identity matmul accumulate)
                _mm(nc, vps, wid_sb, xbf_sb[:, oc, ts(qi, QC)], False, True)
                ot = outp.tile([128, QC], F32)
                if oc == 0:
                    nc.scalar.copy(ot, vps)
                else:
                    nc.vector.tensor_copy(ot, vps)
                nc.sync.dma_start(out=outr[:, oc, ts(qi, QC)], in_=ot)

            # ---- main loop over query chunks ----
            for qi in range(NQC):
                # row 0: softmax denominator (ones column in hT);
                # rows 1-32: x0 channels.
                x0 = px0.tile([CI + 1, QC], F32)
                x0q = []
                for gi, g0 in enumerate(range(0, NKC, GRP)):
                    ps = pl.tile([128, GRP, QC], F32, tag="lg")
                    eT = eTp.tile([128, GRP, QC], BF16)
                    for j in range(GRP):
                        kc = g0 + j
                        # row-packed: strip kc%4 holds its own copy of
                        # g/f, so adjacent matmuls execute concurrently
                        # in different PE row bands.
                        s = kc % 4
                        sl = slice(32 * s, 32 * (s + 1))
                        nc.tensor.matmul(
                            ps[:, j, :],
                            lhsT=g_sb[sl, ts(kc, KC)],
                            rhs=f_sb[sl, ts(qi, QC)],
                            start=True, stop=True,
                            tile_position=(32 * s, 0),
                        )
                    if gi in DVE_GROUPS:
                        # Schraudolph fast-exp on DVE: bf16 bits of
                        # exp(l) ~= int16(l*EXP_A + EXP_B)
                        nc.vector.tensor_scalar(
                            out=eT.bitcast(I16), in0=ps,
                            scalar1=EXP_A, scalar2=EXP_B,
                            op0=MUL, op1=ADD,
                        )
                    else:
                        nc.scalar.activation(out=eT, in_=ps, func=EXP)
                    # software-pipeline the x0 stage: its wait on this
                    # group's exp then overlaps later groups' logits in
                    # the in-order PE stream.
                    x0q.append((g0, eT))
                    if len(x0q) > PIPE:
                        pg0, peT = x0q.pop(0)
                        for j in range(GRP):
                            kc = pg0 + j
                            _mm(nc, x0, hT_sb[:, kc, :], peT[:, j, :],
                                kc == 0, kc == NKC - 1)
                    if gi % FILL_EVERY == FILL_EVERY - 1:
                        fill(1)
                    if qi > 0:
                        if gi == 3:
                            emit_out(qi - 1, 0)
                        elif gi == 6:
                            emit_out(qi - 1, 1)
                            del x0a_by_chunk[qi - 1]
                for pg0, peT in x0q:
                    for j in range(GRP):
                        kc = pg0 + j
                        _mm(nc, x0, hT_sb[:, kc, :], peT[:, j, :],
                            kc == 0, kc == NKC - 1)
                # softmax divide: row 0 of x0 is the denominator
                rcp = smallp.tile([1, QC], F32, tag="rcp")
                nc.vector.reciprocal_approx_fast(out=rcp, in_=x0[0:1, :])
                rcp_b = smallp.tile([CI + 1, QC], F32, tag="rcpb")
                nc.gpsimd.partition_broadcast(rcp_b, rcp)
                x0a = smallp.tile([CI + 1, QC], BF16, tag="x0a")
                nc.vector.tensor_mul(x0a, x0, rcp_b)
                x0a_by_chunk[qi] = x0a
            emit_out(NQC - 1, 0)
            emit_out(NQC - 1, 1)

    nc.compile()
    return nc


def kernel(x, wf, bf, wg, bg, wh, bh, wv, bv, gamma):
    global _cached_nc, LAST_EXEC_NS
    if _cached_nc is None:
        _cached_nc = _build()
    nc = _cached_nc

    x = np.asarray(x, dtype=np.float32)
    wf = np.asarray(wf, dtype=np.float32)
    bf = np.asarray(bf, dtype=np.float32)
    wg = np.asarray(wg, dtype=np.float32)
    bg = np.asarray(bg, dtype=np.float32)
    wh = np.asarray(wh, dtype=np.float32)
    bh = np.asarray(bh, dtype=np.float32)
    wv = np.asarray(wv, dtype=np.float32)
    bv = np.asarray(bv, dtype=np.float32)
    g0 = float(np.asarray(gamma, dtype=np.float32).reshape(-1)[0])

    bf16 = ml_dtypes.bfloat16
    xf = np.ascontiguousarray(x.reshape(B, C, N))
    # f/g weights replicated 4x along M so f/g land replicated on the
    # four 32-partition strips (enables row-packed logits matmuls).
    wfT = np.ascontiguousarray(np.tile(wf.T, (1, 4))).astype(bf16)
    wgT = np.ascontiguousarray(np.tile(wg.T, (1, 4))).astype(bf16)
    whT = np.ascontiguousarray(wh.T).astype(bf16)
    wvT = np.empty((CI + 1, C), np.float32)              # aug: bias row 0
    wvT[0, :] = g0 * (bv + wv @ bh)
    wvT[1:, :] = g0 * wv.T
    wvT = wvT.astype(bf16)
    wid = np.eye(128, dtype=np.float32).astype(bf16)
    bf4 = np.ascontiguousarray(np.tile(bf, 4).reshape(128, 1))
    bg4 = np.ascontiguousarray(np.tile(bg, 4).reshape(128, 1))

    in_maps = []
    for core in range(NCORES):
        b, half = divmod(core, 2)
        xb = xf[b]
        if half:
            xb = np.concatenate([xb[:, NQ:], xb[:, :NQ]], axis=1)
        in_maps.append(
            {"xbf": np.ascontiguousarray(xb.astype(bf16)), "wfT": wfT,
             "wgT": wgT, "whT": whT, "wvT": wvT, "wid": wid,
             "bf": bf4, "bg": bg4}
        )

    res = run_bass_kernel_spmd(
        nc, in_maps, list(range(NCORES)),
        trace=TRACE or bool(os.environ.get("BASS_KERNEL_TRACE")),
    )
    LAST_EXEC_NS = res.exec_time_ns

    out = np.empty((B, C, N), np.float32)
    for core in range(NCORES):
        b, half = divmod(core, 2)
        out[b][:, half * NQ : (half + 1) * NQ] = res.results[core]["out"]
    return out.reshape(B, C, W, H)


# revision 8
# speedup vs baseline: 1.5277x; 1.1312x over previous
"""Trainium2 Bass kernel for the attention layer:

    f = wf@x+bf; g = wg@x+bg; h = wh@x+bh            (1x1 convs, Ci=32)
    attn = softmax(f^T g, axis=-1)                   (per batch, N=4096)
    out = (wv @ (h @ attn^T) + bv) * gamma + x

Sharding: 8 cores = 4 batches x 2 query-halves (2048 queries each).
Each core receives the full (256, 4096) batch slice with its query half
permuted to the front, so the SPMD program uses fixed offsets.

Per-core dataflow (all-bf16 matmuls, PSUM fp32 accumulate):
  - x arrives bf16 only (2MB); the residual is added from the bf16 copy
    via an IDENTITY MATMUL accumulated into the projection PSUM bank,
    so the output copy is a pure PSUM->SBUF copy (balanceable between
    ACT and DVE) instead of a DVE-only tensor_tensor add.
  - exp is the elementwise bottleneck (32*2048 partition-cycles/core)
    and GPSIMD/DMA cannot touch PSUM, so every PSUM consumer is either
    ACT or DVE: exp groups alternate ACT (true exp, PSUM->bf16) and DVE
    (Schraudolph fast-exp: bf16 bits = int16(l*128/ln2 + 16250) in one
    tensor_scalar, ~3.5% rel err); PSUM->SBUF copies alternate engines;
    softmax divide uses reciprocal_approx_fast.
  - PSUM budget (8 banks): logits pool 3 bufs x 2 banks so the logits
    matmul for group g WAR-waits exp(g-3), letting both exp engines run
    fully parallel; 1 bank for projections/warmup/fillers; 1 bank for
    the x0 accumulator.
  - the whole f/g/hT prologue runs before the chunk loop, cycling
    through the logits pool buffers; hT lands 16 key-chunks per PSUM
    bank ([128,16,32] fp32 = 2KB) so one big copy replaces 16 small.
  - per 512-query chunk: 32 row-packed logits matmuls (strip kc%4,
    concurrent PE row bands) -> exp groups of 2 k-chunks -> 32
    x0-accumulation matmuls (hT stationary, ones column in row 0
    accumulating the softmax denominator).
  - projection+residual+DMA for chunk i are DEFERRED into chunk i+1's
    group loop (two halves at different groups so PSUM WARs stay off
    the PE's critical path); dummy 512-row matmuls pad PE idle slots so
    the HAM clock gate (K/N pulse gating, 1.2 vs 2.4 GHz) stays open.
"""

import os
import numpy as np
import ml_dtypes

import concourse.bass as bass
import concourse.mybir as mybir
import concourse.tile as tile
from concourse import bacc
from concourse.bass import ts
from concourse.bass_utils import run_bass_kernel_spmd

F32 = mybir.dt.float32
F32R = mybir.dt.float32r
BF16 = mybir.dt.bfloat16
I16 = mybir.dt.int16
EXP = mybir.ActivationFunctionType.Exp
IDENT = mybir.ActivationFunctionType.Identity
MUL = mybir.AluOpType.mult
ADD = mybir.AluOpType.add

B, C, W, H = 4, 256, 64, 64
N = W * H            # 4096 keys/queries per batch
CI = 32              # inner channels
NCORES = 8
NQ = N // 2          # queries per core
QC = 512             # query chunk = one fp32 PSUM bank
NQC = NQ // QC       # 4 query chunks per core
KC = 128             # key chunk = partition dim
NKC = N // KC        # 32 key chunks
GRP = 2              # key chunks per exp group (PSUM banks per tile)
NGRP = NKC // GRP    # 16 groups per chunk
PIPE = 2             # software-pipeline depth (groups) for x0 stage
NWARM = 4            # dummy bf16 matmuls to warm the PE clock gate
FILL_EVERY = 8       # PE filler matmul every this many groups
NSLICE = 4           # x DMA slices

# Schraudolph fast-exp constants (bf16 bits = int16(l*EXP_A + EXP_B))
EXP_A = 128.0 / float(np.log(2.0))
EXP_B = 16250.0
# groups handled by DVE fast-exp (rest go to ACT true exp): 7 of 16
DVE_GROUPS = frozenset({1, 3, 5, 7, 9, 11, 13})

# Trace knob for test harnesses: set kernel.TRACE = True to profile.
TRACE = False
LAST_EXEC_NS = None

_cached_nc = None


def _mm(nc, out, lhsT, rhs, start, stop, tile_position=None):
    nc.tensor.matmul(out, lhsT=lhsT, rhs=rhs, start=start, stop=stop,
                     tile_position=tile_position)


def _build():
    nc = bacc.Bacc(
        "TRN2", target_bir_lowering=False, debug=False, num_devices=NCORES
    )
    xbf_d = nc.dram_tensor("xbf", (C, N), BF16, kind="ExternalInput").ap()
    wfT_d = nc.dram_tensor("wfT", (C, 128), BF16, kind="ExternalInput").ap()
    wgT_d = nc.dram_tensor("wgT", (C, 128), BF16, kind="ExternalInput").ap()
    whT_d = nc.dram_tensor("whT", (C, CI), BF16, kind="ExternalInput").ap()
    wvT_d = nc.dram_tensor("wvT", (CI + 1, C), BF16, kind="ExternalInput").ap()
    wid_d = nc.dram_tensor("wid", (128, 128), BF16, kind="ExternalInput").ap()
    bf_d = nc.dram_tensor("bf", (128, 1), F32, kind="ExternalInput").ap()
    bg_d = nc.dram_tensor("bg", (128, 1), F32, kind="ExternalInput").ap()
    out_d = nc.dram_tensor("out", (C, NQ), F32, kind="ExternalOutput").ap()

    outr = out_d.rearrange("(oc p) n -> p oc n", p=128)
    xbfr = xbf_d.rearrange("(cc p) n -> p cc n", p=128)

    with tile.TileContext(nc) as tc:
        with (
            tc.tile_pool(name="consts", bufs=1) as consts,
            tc.tile_pool(name="data", bufs=1) as data,
            tc.tile_pool(name="eTp", bufs=6) as eTp,
            tc.tile_pool(name="smallp", bufs=2) as smallp,
            tc.tile_pool(name="outp", bufs=3) as outp,
            tc.tile_pool(name="pl", bufs=3, space="PSUM") as pl,
            tc.tile_pool(name="pp", bufs=1, space="PSUM") as pp,
            tc.tile_pool(name="px0", bufs=1, space="PSUM") as px0,
        ):
            # ---- PE + ACT warm-up (overlaps the input DMAs) ----
            scratch = consts.tile([128, QC], BF16)
            nc.vector.memset(scratch, 0.0)

            def fill(n=1):
                # HAM filler: keeps the PE streaming through slots where
                # it would otherwise idle (and re-throttle to 1.2 GHz).
                wps = pp.tile([128, QC], F32, tag="pp", name="wps")
                for _ in range(n):
                    nc.tensor.matmul(
                        wps, lhsT=scratch[:, 0:128], rhs=scratch,
                        start=True, stop=True, skip_group_check=True,
                    )

            fill(NWARM)
            scratch2 = consts.tile([1, 8], F32)
            nc.scalar.activation(out=scratch2, in_=scratch[0:1, 0:8], func=EXP)

            # ---- constants ----
            wfT_sb = consts.tile([128, 2, 128], BF16)
            nc.sync.dma_start(
                out=wfT_sb, in_=wfT_d.rearrange("(cc p) o -> p cc o", p=128)
            )
            wgT_sb = consts.tile([128, 2, 128], BF16)
            nc.sync.dma_start(
                out=wgT_sb, in_=wgT_d.rearrange("(cc p) o -> p cc o", p=128)
            )
            whT_sb = consts.tile([128, 2, CI], BF16)
            nc.sync.dma_start(
                out=whT_sb, in_=whT_d.rearrange("(cc p) o -> p cc o", p=128)
            )
            wvT_sb = consts.tile([CI + 1, 2, 128], BF16)
            nc.sync.dma_start(
                out=wvT_sb, in_=wvT_d.rearrange("p (oc m) -> p oc m", oc=2)
            )
            wid_sb = consts.tile([128, 128], BF16)
            nc.sync.dma_start(out=wid_sb, in_=wid_d)
            bf_sb = consts.tile([128, 1], F32)
            nc.sync.dma_start(out=bf_sb, in_=bf_d)
            bg_sb = consts.tile([128, 1], F32)
            nc.sync.dma_start(out=bg_sb, in_=bg_d)
            ones_sb = consts.tile([128, 1], BF16)
            nc.vector.memset(ones_sb, 1.0)

            # ---- x (bf16, 4 column slices so compute starts early) ----
            xbf_sb = data.tile([128, 2, N], BF16)
            for s in range(NSLICE):
                nc.sync.dma_start(
                    out=xbf_sb[:, :, ts(s, N // NSLICE)],
                    in_=xbfr[:, :, ts(s, N // NSLICE)],
                )

            # ---- f, g (replicated on 4 strips), hT ----
            f_sb = data.tile([128, NQ], BF16)
            g_sb = data.tile([128, N], BF16)
            hT_sb = data.tile([128, NKC, CI + 1], BF16)
            nc.vector.tensor_copy(
                hT_sb[:, :, 0:1], ones_sb.to_broadcast([128, NKC, 1])
            )

            # prologue blocks cycle through the logits pool's 3 buffers;
            # PSUM->SBUF copy work alternates between ACT and DVE.
            def emit_fg(dst, w_sb, b_sb, j, on_act):
                ps = pl.tile([128, GRP, QC], F32, tag="lg", name=f"fg{j}")
                for cc in range(2):
                    _mm(nc, ps[:, 0, :], w_sb[:, cc, :],
                        xbf_sb[:, cc, ts(j, QC)], cc == 0, cc == 1)
                if on_act:
                    nc.scalar.activation(
                        out=dst[:, ts(j, QC)], in_=ps[:, 0, :], func=IDENT,
                        bias=b_sb,
                    )
                else:
                    nc.vector.tensor_scalar_add(
                        dst[:, ts(j, QC)], ps[:, 0, :], b_sb
                    )

            # hT production: one PSUM bank holds 16 key-chunks
            # ([128,16,32] fp32 = 2KB/partition), one big copy each.
            def emit_hT(half, on_act):
                hps = pl.tile([128, 16, CI], F32, tag="lg", name=f"ph{half}")
                for sl in range(16):
                    kc = 16 * half + sl
                    for cc in range(2):
                        _mm(nc, hps[:, sl, :], xbf_sb[:, cc, ts(kc, KC)],
                            whT_sb[:, cc, :], cc == 0, cc == 1)
                dst = hT_sb[:, 16 * half : 16 * (half + 1), 1 : CI + 1]
                if on_act:
                    nc.scalar.copy(dst, hps)
                else:
                    nc.vector.tensor_copy(dst, hps)

            # ---- prologue: f (own queries), g + hT (all keys) ----
            for j in range(4):
                emit_fg(f_sb, wfT_sb, bf_sb, j, on_act=(j % 2 == 0))
                emit_fg(g_sb, wgT_sb, bg_sb, j, on_act=(j % 2 == 1))
                fill(1)
            emit_hT(0, on_act=True)
            for j in range(4, 8):
                emit_fg(g_sb, wgT_sb, bg_sb, j, on_act=(j % 2 == 1))
                fill(1)
            emit_hT(1, on_act=False)

            # deferred projection + residual + output for chunk qi
            x0a_by_chunk = {}

            def emit_out(qi, oc, tail=False):
                x0a = x0a_by_chunk[qi]
                if tail and oc == 1:
                    # final chunk: second projection borrows a logits
                    # bank so both output copies run concurrently.
                    big = pl.tile([128, GRP, QC], F32, tag="lg", name="pst")
                    vps = big[:, 0, :]
                else:
                    vps = pp.tile([128, QC], F32, tag="pp",
                                  name=f"psv{qi}{oc}")
                _mm(nc, vps, wvT_sb[:, oc, :], x0a, True, False)
                # residual: + I @ x  (identity matmul accumulate)
                _mm(nc, vps, wid_sb, xbf_sb[:, oc, ts(qi, QC)], False, True)
                ot = outp.tile([128, QC], F32)
                if oc == 0:
                    nc.scalar.copy(ot, vps)
                else:
                    nc.vector.tensor_copy(ot, vps)
                nc.sync.dma_start(out=outr[:, oc, ts(qi, QC)], in_=ot)

            # ---- main loop over query chunks ----
            for qi in range(NQC):
                # row 0: softmax denominator (ones column in hT);
                # rows 1-32: x0 channels.
                x0 = px0.tile([CI + 1, QC], F32)
                x0q = []
                for gi, g0 in enumerate(range(0, NKC, GRP)):
                    ps = pl.tile([128, GRP, QC], F32, tag="lg")
                    eT = eTp.tile([128, GRP, QC], BF16)
                    for j in range(GRP):
                        kc = g0 + j
                        # row-packed: strip kc%4 holds its own copy of
                        # g/f, so adjacent matmuls execute concurrently
                        # in different PE row bands.
                        s = kc % 4
                        sl = slice(32 * s, 32 * (s + 1))
                        nc.tensor.matmul(
                            ps[:, j, :],
                            lhsT=g_sb[sl, ts(kc, KC)],
                            rhs=f_sb[sl, ts(qi, QC)],
                            start=True, stop=True,
                            tile_position=(32 * s, 0),
                        )
                    if gi in DVE_GROUPS:
                        # Schraudolph fast-exp on DVE: bf16 bits of
                        # exp(l) ~= int16(l*EXP_A + EXP_B)
                        nc.vector.tensor_scalar(
                            out=eT.bitcast(I16), in0=ps,
                            scalar1=EXP_A, scalar2=EXP_B,
                            op0=MUL, op1=ADD,
                        )
                    else:
                        nc.scalar.activation(out=eT, in_=ps, func=EXP)
                    # software-pipeline the x0 stage: its wait on this
                    # group's exp then overlaps later groups' logits in
                    # the in-order PE stream.
                    x0q.append((g0, eT))
                    if len(x0q) > PIPE:
                        pg0, peT = x0q.pop(0)
                        for j in range(GRP):
                            kc = pg0 + j
                            _mm(nc, x0, hT_sb[:, kc, :], peT[:, j, :],
                                kc == 0, kc == NKC - 1)
                    if gi % FILL_EVERY == FILL_EVERY - 1:
                        fill(1)
                    if qi > 0:
                        if gi == 3:
                            emit_out(qi - 1, 0)
                        elif gi == 6:
                            emit_out(qi - 1, 1)
                            del x0a_by_chunk[qi - 1]
                for pg0, peT in x0q:
                    for j in range(GRP):
                        kc = pg0 + j
                        _mm(nc, x0, hT_sb[:, kc, :], peT[:, j, :],
                            kc == 0, kc == NKC - 1)
                # softmax divide: row 0 of x0 is the denominator
                rcp = smallp.tile([1, QC], F32, tag="rcp")
                nc.vector.reciprocal_approx_fast(out=rcp, in_=x0[0:1, :])
                rcp_b = smallp.tile([CI + 1, QC], F32, tag="rcpb")
                nc.gpsimd.partition_broadcast(rcp_b, rcp)
                x0a = smallp.tile([CI + 1, QC], BF16, tag="x0a")
                nc.vector.tensor_mul(x0a, x0, rcp_b)
                x0a_by_chunk[qi] = x0a
            emit_out(NQC - 1, 0, tail=True)
            emit_out(NQC - 1, 1, tail=True)

    nc.compile()
    return nc


def kernel(x, wf, bf, wg, bg, wh, bh, wv, bv, gamma):
    global _cached_nc, LAST_EXEC_NS
    if _cached_nc is None:
        _cached_nc = _build()
    nc = _cached_nc

    x = np.asarray(x, dtype=np.float32)
    wf = np.asarray(wf, dtype=np.float32)
    bf = np.asarray(bf, dtype=np.float32)
    wg = np.asarray(wg, dtype=np.float32)
    bg = np.asarray(bg, dtype=np.float32)
    wh = np.asarray(wh, dtype=np.float32)
    bh = np.asarray(bh, dtype=np.float32)
    wv = np.asarray(wv, dtype=np.float32)
    bv = np.asarray(bv, dtype=np.float32)
    g0 = float(np.asarray(gamma, dtype=np.float32).reshape(-1)[0])

    bf16 = ml_dtypes.bfloat16
    xf = np.ascontiguousarray(x.reshape(B, C, N))
    # f/g weights replicated 4x along M so f/g land replicated on the
    # four 32-partition strips (enables row-packed logits matmuls).
    wfT = np.ascontiguousarray(np.tile(wf.T, (1, 4))).astype(bf16)
    wgT = np.ascontiguousarray(np.tile(wg.T, (1, 4))).astype(bf16)
    whT = np.ascontiguousarray(wh.T).astype(bf16)
    wvT = np.empty((CI + 1, C), np.float32)              # aug: bias row 0
    wvT[0, :] = g0 * (bv + wv @ bh)
    wvT[1:, :] = g0 * wv.T
    wvT = wvT.astype(bf16)
    wid = np.eye(128, dtype=np.float32).astype(bf16)
    bf4 = np.ascontiguousarray(np.tile(bf, 4).reshape(128, 1))
    bg4 = np.ascontiguousarray(np.tile(bg, 4).reshape(128, 1))

    in_maps = []
    for core in range(NCORES):
        b, half = divmod(core, 2)
        xb = xf[b]
        if half:
            xb = np.concatenate([xb[:, NQ:], xb[:, :NQ]], axis=1)
        in_maps.append(
            {"xbf": np.ascontiguousarray(xb.astype(bf16)), "wfT": wfT,
             "wgT": wgT, "whT": whT, "wvT": wvT, "wid": wid,
             "bf": bf4, "bg": bg4}
        )

    res = run_bass_kernel_spmd(
        nc, in_maps, list(range(NCORES)),
        trace=TRACE or bool(os.environ.get("BASS_KERNEL_TRACE")),
    )
    LAST_EXEC_NS = res.exec_time_ns

    out = np.empty((B, C, N), np.float32)
    for core in range(NCORES):
        b, half = divmod(core, 2)
        out[b][:, half * NQ : (half + 1) * NQ] = res.results[core]["out"]
    return out.reshape(B, C, W, H)
